# revision 10
# baseline (speedup 1.0000x reference)
"""DeformConv1d Bass kernel for Trainium2 (8 NeuronCores, data-parallel over batch).

Math (G=DG=1, K=3, stride=1, pad=1, dil=1):
  offset/mask branches: depthwise(k=7,pad=3) -> pointwise(1x1) convs, fused host-side
  into one 7-tap 64->6ch conv: y[f,l] = sum_{tau,c} Wf[f,c,tau] x[c,l+tau-3] + beff[f].
  Linear interpolation at p = l+k-1+off equals a 3-tap tent MAC for |off|<1:
    val_k[c,l] = msk_k[l]*( relu(-off_k)[l]*x[c,l+k-2]
               + (1-|off_k|)[l]*x[c,l+k-1] + relu(off_k)[l]*x[c,l+k] )
  out[o,l] = sum_{c,k} weight[o,c,k] val_k[c,l] + bias[o]
|off| ~ 0.05 here; the |off|<1 assumption is checked in test.py.

Layout: per core 2 batches. Products run in an "R32" layout:
  partition r = b*64 + l_blk*32 + c_hi   (b in 0..1, l_blk in 0..1, c_hi in 0..31)
  free = (c_lo in 0..1, l_in)            c = c_hi*2 + c_lo, l = l_blk*8192 + l_in
so the per-(k,d) modulation row only needs 32x partition replication (DRAM-broadcast
DMA) and broadcasts over c_lo via a stride-0 free dim. The (c,k) contraction then
runs as 6 accumulated p=32 matmuls per (b, l_blk) group - the four groups occupy
different 32-row PE groups and execute concurrently.
"""

import numpy as np
import ml_dtypes

import concourse.bass as bass
import concourse.bacc as bacc
import concourse.mybir as mybir
from concourse.tile import TileContext
from concourse.bass_utils import run_bass_kernel_spmd

B, C, CO, L, K = 16, 64, 64, 16384, 3
NCORES = 8
BLOC = B // NCORES          # 2 batches per core
HALO = 8
LP = L + 2 * HALO
MMN = 512                   # matmul free-dim chunk (one PSUM bank of fp32)
SEG = 128                   # smallside seg-layout: l = p*SEG + j
LB = L // 2                 # l_blk size (8192)
LBH = LB + 2 * HALO         # halo'd l_in extent per l_blk row
LC = 1024                   # product chunk along l_in
BF = mybir.dt.bfloat16
F32 = mybir.dt.float32
BF_NP = ml_dtypes.bfloat16

_BUILD_CACHE = {}


def _build():
    if "nc" in _BUILD_CACHE:
        return _BUILD_CACHE["nc"]
    nc = bacc.Bacc("TRN2")

    x2h = nc.dram_tensor("x2h", [2 * C, LP], BF, kind="ExternalInput")
    x2r = nc.dram_tensor("x2r", [128, 2 * LBH], BF, kind="ExternalInput")
    w_br = nc.dram_tensor("w_br", [2 * C, 7 * 6], BF, kind="ExternalInput")
    b_br = nc.dram_tensor("b_br", [128, 6], F32, kind="ExternalInput")
    w_vr = nc.dram_tensor("w_vr", [128, 2 * K * CO], BF, kind="ExternalInput")
    b_out = nc.dram_tensor("b_out", [CO, 1], F32, kind="ExternalInput")
    out = nc.dram_tensor("out", [BLOC, CO, L], F32, kind="ExternalOutput")

    br_dram = nc.dram_tensor("br_scratch", [BLOC, 6, L], F32)
    mw_dram = nc.dram_tensor("mw_scratch", [BLOC, 9, L], BF)

    with TileContext(nc) as tc:
        with (
            tc.tile_pool(name="outer", bufs=1) as bigp,
            tc.tile_pool(name="const", bufs=1) as constp,
            tc.tile_pool(name="osb", bufs=3) as outp,
        ):
            x2r_sb = bigp.tile([128, 2 * LBH], BF, tag="x2r")
            nc.sync.dma_start(out=x2r_sb[:], in_=x2r[:])
            wvr_sb = constp.tile([128, 2 * K * CO], BF, tag="wvr")
            nc.sync.dma_start(out=wvr_sb[:], in_=w_vr[:])
            bout_sb = constp.tile([CO, 1], F32, tag="bout")
            nc.sync.dma_start(out=bout_sb[:], in_=b_out[:])

            # ======== phase 1: branch convs + smallside (scoped pools) ========
            with (
                tc.tile_pool(name="ph1", bufs=1) as p1p,
                tc.tile_pool(name="seg", bufs=1) as segp,
                tc.tile_pool(name="ps_br", bufs=3, space="PSUM") as psbrp,
            ):
                xsb = p1p.tile([2 * C, LP], BF, tag="x")
                nc.sync.dma_start(out=xsb[:], in_=x2h[:])
                wbr_sb = constp.tile([2 * C, 7 * 6], BF, tag="wbr")
                nc.sync.dma_start(out=wbr_sb[:], in_=w_br[:])
                bbr_sb = constp.tile([128, 6], F32, tag="bbr")
                nc.sync.dma_start(out=bbr_sb[:], in_=b_br[:])

                for i in range(L // MMN):
                    pss = [psbrp.tile([6, MMN], F32, tag=f"psbr{b}", name=f"psbr{b}")
                           for b in range(BLOC)]
                    for t in range(7):
                        for b in range(BLOC):
                            nc.tensor.matmul(
                                pss[b][:],
                                lhsT=wbr_sb[b * C:(b + 1) * C, t * 6:(t + 1) * 6],
                                rhs=xsb[b * C:(b + 1) * C,
                                        HALO + i * MMN + (t - 3): HALO + (i + 1) * MMN + (t - 3)],
                                start=(t == 0), stop=(t == 6),
                            )
                    for b in range(BLOC):
                        bro = outp.tile([6, MMN], F32, tag=f"bro{b}", name=f"bro{b}")
                        if b == 0:
                            nc.vector.tensor_copy(out=bro[:], in_=pss[b][:])
                        else:
                            nc.scalar.copy(out=bro[:], in_=pss[b][:])
                        nc.sync.dma_start(out=br_dram[b, :, i * MMN:(i + 1) * MMN],
                                          in_=bro[:])

                # ---- smallside in seg layout [128, (b, f, j)] ----
                nseg = L // SEG  # 128
                brseg = segp.tile([nseg, BLOC * 6 * SEG], F32, tag="brseg")
                nc.sync.dma_start(
                    out=brseg[:].rearrange("p (b f j) -> p b f j", b=BLOC, f=6),
                    in_=br_dram[:].rearrange("b f (p j) -> p b f j", j=SEG),
                )
                brv = brseg[:].rearrange("p (b f j) -> p b f j", b=BLOC, f=6)
                nc.vector.tensor_tensor(
                    out=brv, in0=brv,
                    in1=bbr_sb[:].unsqueeze(1).unsqueeze(3).to_broadcast(
                        (nseg, BLOC, 6, SEG)),
                    op=mybir.AluOpType.add,
                )
                mskseg = segp.tile([nseg, BLOC * 3 * SEG], F32, tag="mskseg")
                am1 = segp.tile([nseg, BLOC * 3 * SEG], F32, tag="am1")
                ap1 = segp.tile([nseg, BLOC * 3 * SEG], F32, tag="ap1")
                a0 = segp.tile([nseg, BLOC * 3 * SEG], F32, tag="a0")
                for b in range(BLOC):
                    dst = slice(b * 3 * SEG, (b + 1) * 3 * SEG)
                    nc.scalar.activation(
                        out=mskseg[:, dst].rearrange("p (g j) -> p g j", j=SEG),
                        in_=brv[:, b, 3:6, :],
                        func=mybir.ActivationFunctionType.Sigmoid)
                    nc.scalar.activation(
                        out=am1[:, dst].rearrange("p (g j) -> p g j", j=SEG),
                        in_=brv[:, b, 0:3, :],
                        func=mybir.ActivationFunctionType.Relu, scale=-1.0)
                    nc.scalar.activation(
                        out=ap1[:, dst].rearrange("p (g j) -> p g j", j=SEG),
                        in_=brv[:, b, 0:3, :],
                        func=mybir.ActivationFunctionType.Relu)
                nc.vector.tensor_tensor(out=a0[:], in0=am1[:], in1=ap1[:],
                                        op=mybir.AluOpType.add)
                nc.vector.tensor_scalar(out=a0[:], in0=a0[:], scalar1=-1.0,
                                        scalar2=1.0, op0=mybir.AluOpType.mult,
                                        op1=mybir.AluOpType.add)
                mwseg = segp.tile([nseg, BLOC * 9 * SEG], BF, tag="mwseg")
                mwv = mwseg[:].rearrange("p (b d g) -> p b d g", b=BLOC, d=3)
                for b in range(BLOC):
                    sl = slice(b * 3 * SEG, (b + 1) * 3 * SEG)
                    for d_idx, t in enumerate((am1, a0, ap1)):
                        nc.vector.tensor_tensor(out=mwv[:, b, d_idx, :],
                                                in0=mskseg[:, sl], in1=t[:, sl],
                                                op=mybir.AluOpType.mult)
                nc.sync.dma_start(
                    out=mw_dram[:].rearrange("b f (p j) -> p b f j", j=SEG),
                    in_=mwseg[:].rearrange("p (b f j) -> p b f j", b=BLOC, f=9),
                )

            # ======== phase 2: products (R32 layout) + val matmuls ========
            ph2 = (
                tc.tile_pool(name="mw", bufs=2),
                tc.tile_pool(name="val", bufs=2),
                tc.tile_pool(name="tmp", bufs=2),
                tc.tile_pool(name="ps_o", bufs=2, space="PSUM"),
            )
            mwp, valp, tmpp, psop = [p.__enter__() for p in ph2]
            for ci in range(LB // LC):
                j0 = HALO + ci * LC
                mw2 = []
                for f in range(9):
                    t = mwp.tile([128, LC], BF, tag=f"mw2_{f}", name=f"mw2_{f}")
                    for b in range(BLOC):
                        for g in range(2):
                            nc.sync.dma_start(
                                out=t[b * 64 + g * 32: b * 64 + (g + 1) * 32, :],
                                in_=mw_dram[b, f:f + 1,
                                            g * LB + ci * LC: g * LB + (ci + 1) * LC
                                            ].partition_broadcast(32),
                            )
                    mw2.append(t)

                def xv(shift):
                    return x2r_sb[:].rearrange("r (c j) -> r c j", c=2)[
                        :, :, j0 + shift: j0 + shift + LC]

                vals = []
                for k in range(K):
                    vk = valp.tile([128, 2 * LC], BF, tag=f"val_{k}", name=f"val_{k}")
                    vkv = vk[:].rearrange("r (c j) -> r c j", c=2)
                    tmp = tmpp.tile([128, 2 * LC], BF, tag="ptmp", name="ptmp")
                    tmpv = tmp[:].rearrange("r (c j) -> r c j", c=2)
                    nc.vector.tensor_tensor(
                        out=vkv, in0=xv(k - 2),
                        in1=mw2[0 * 3 + k][:].unsqueeze(1).to_broadcast((128, 2, LC)),
                        op=mybir.AluOpType.mult)
                    nc.vector.tensor_tensor(
                        out=tmpv, in0=xv(k - 1),
                        in1=mw2[1 * 3 + k][:].unsqueeze(1).to_broadcast((128, 2, LC)),
                        op=mybir.AluOpType.mult)
                    nc.vector.tensor_tensor(out=vk[:], in0=vk[:], in1=tmp[:],
                                            op=mybir.AluOpType.add)
                    nc.vector.tensor_tensor(
                        out=tmpv, in0=xv(k),
                        in1=mw2[2 * 3 + k][:].unsqueeze(1).to_broadcast((128, 2, LC)),
                        op=mybir.AluOpType.mult)
                    nc.vector.tensor_tensor(out=vk[:], in0=vk[:], in1=tmp[:],
                                            op=mybir.AluOpType.add)
                    vals.append(vk)

                for nh in range(LC // MMN):
                    psos = [psop.tile([CO, MMN], F32, tag=f"pso{g}", name=f"pso{g}")
                            for g in range(4)]
                    for k in range(K):
                        for cl in range(2):
                            for g in range(4):
                                rb = g * 32
                                nc.tensor.matmul(
                                    psos[g][:],
                                    lhsT=wvr_sb[rb:rb + 32,
                                                (cl * K + k) * CO:(cl * K + k + 1) * CO],
                                    rhs=vals[k][rb:rb + 32,
                                                cl * LC + nh * MMN: cl * LC + (nh + 1) * MMN],
                                    start=(k == 0 and cl == 0),
                                    stop=(k == K - 1 and cl == 1),
                                    tile_position=(rb, 0),
                                )
                    for g in range(4):
                        b, lb = g // 2, g % 2
                        osb = outp.tile([CO, MMN], F32, tag=f"osb{g}", name=f"osb{g}")
                        nc.scalar.activation(out=osb[:], in_=psos[g][:],
                                             func=mybir.ActivationFunctionType.Identity,
                                             bias=bout_sb[:], scale=1.0)
                        l0 = lb * LB + ci * LC + nh * MMN
                        nc.sync.dma_start(out=out[b, :, l0:l0 + MMN], in_=osb[:])

            for p in reversed(ph2):
                p.__exit__(None, None, None)

    nc.compile()
    _BUILD_CACHE["nc"] = nc
    return nc


def _host_prep(inputs):
    x = np.asarray(inputs["x"], np.float32)
    w_off_dw = np.asarray(inputs["w_off_dw"], np.float32)
    b_off_dw = np.asarray(inputs["b_off_dw"], np.float32)
    w_off_pw = np.asarray(inputs["w_off_pw"], np.float32)
    b_off_pw = np.asarray(inputs["b_off_pw"], np.float32)
    w_msk_dw = np.asarray(inputs["w_msk_dw"], np.float32)
    b_msk_dw = np.asarray(inputs["b_msk_dw"], np.float32)
    w_msk_pw = np.asarray(inputs["w_msk_pw"], np.float32)
    b_msk_pw = np.asarray(inputs["b_msk_pw"], np.float32)
    weight = np.asarray(inputs["weight"], np.float32)
    bias = np.asarray(inputs["bias"], np.float32)

    wf = np.zeros((C, 7, 6), np.float32)
    wf[:, :, 0:3] = (w_off_pw[:, :, 0].T[:, None, :] * w_off_dw[:, 0, :][:, :, None])
    wf[:, :, 3:6] = (w_msk_pw[:, :, 0].T[:, None, :] * w_msk_dw[:, 0, :][:, :, None])
    beff = np.zeros(6, np.float32)
    beff[0:3] = b_off_pw + w_off_pw[:, :, 0] @ b_off_dw
    beff[3:6] = b_msk_pw + w_msk_pw[:, :, 0] @ b_msk_dw

    w_br = np.vstack([wf.reshape(C, 42)] * 2).astype(BF_NP)
    b_br = np.broadcast_to(beff[None, :], (128, 6)).copy()
    # w_vr[r, (c_lo, k, o)] = weight[o, c_hi*2+c_lo, k], r = b*64+l_blk*32+c_hi
    wv = weight.transpose(1, 2, 0).reshape(32, 2, K, CO)      # [c_hi, c_lo, k, o]
    wv = wv.reshape(32, 2 * K * CO)                           # [c_hi, (c_lo k o)]
    w_vr = np.vstack([wv] * 4).astype(BF_NP)                  # 4 row-groups
    b_out = bias.reshape(CO, 1).copy()

    in_maps = []
    for core in range(NCORES):
        xb = x[core * BLOC:(core + 1) * BLOC]                 # [2, C, L]
        xbf = xb.astype(BF_NP)
        x2h = np.zeros((2 * C, LP), BF_NP)
        x2h[:, HALO:HALO + L] = xbf.reshape(2 * C, L)
        # x2r[b, l_blk, c_hi, c_lo, j] = x[b, c_hi*2+c_lo, l_blk*LB + j - HALO]
        x2r = np.zeros((2, 2, 32, 2, LBH), BF_NP)
        xr = xbf.reshape(2, 32, 2, 2, LB)                     # [b, c_hi, c_lo, l_blk, l_in]
        xr = xr.transpose(0, 3, 1, 2, 4)                      # [b, l_blk, c_hi, c_lo, l_in]
        x2r[:, :, :, :, HALO:HALO + LB] = xr
        x2r[:, 1, :, :, :HALO] = xr[:, 0, :, :, LB - HALO:]
        x2r[:, 0, :, :, HALO + LB:] = xr[:, 1, :, :, :HALO]
        in_maps.append({
            "x2h": x2h, "x2r": x2r.reshape(128, 2 * LBH),
            "w_br": w_br, "b_br": b_br, "w_vr": w_vr, "b_out": b_out,
        })
    return in_maps


def kernel(**inputs):
    nc = _build()
    in_maps = _host_prep(inputs)
    res = run_bass_kernel_spmd(nc, in_maps, list(range(NCORES)))
    out = np.empty((B, CO, L), np.float32)
    for core in range(NCORES):
        out[core * BLOC:(core + 1) * BLOC] = res.results[core]["out"]
    return out


# revision 15
# speedup vs baseline: 1.1903x; 1.1903x over previous
"""DeformConv1d Bass kernel for Trainium2 (8 NeuronCores, data-parallel over batch).

Math (G=DG=1, K=3, stride=1, pad=1, dil=1):
  offset/mask branches: depthwise(k=7,pad=3) -> pointwise(1x1) convs, fused host-side
  into one 7-tap 64->6ch conv: y[f,l] = sum_{tau,c} Wf[f,c,tau] x[c,l+tau-3] + beff[f].
  Linear interpolation at p = l+k-1+off equals a 3-tap tent MAC for |off|<1:
    val_k[c,l] = msk_k[l]*( relu(-off_k)[l]*x[c,l+k-2]
               + (1-|off_k|)[l]*x[c,l+k-1] + relu(off_k)[l]*x[c,l+k] )
  out[o,l] = sum_{c,k} weight[o,c,k] val_k[c,l] + bias[o]
|off| ~ 0.05 here; the |off|<1 assumption is checked in test.py.

Layout: per core 2 batches. Products run in an "R32" layout:
  partition r = b*64 + l_blk*32 + c_hi   (b in 0..1, l_blk in 0..1, c_hi in 0..31)
  free = (c_lo in 0..1, l_in)            c = c_hi*2 + c_lo, l = l_blk*8192 + l_in
so the per-(k,d) modulation row only needs 32x partition replication (DRAM-broadcast
DMA) and broadcasts over c_lo via a stride-0 free dim. The (c,k) contraction then
runs as 6 accumulated p=32 matmuls per (b, l_blk) group - the four groups occupy
different 32-row PE groups and execute concurrently.
"""

import numpy as np
import ml_dtypes

import concourse.bass as bass
import concourse.bacc as bacc
import concourse.mybir as mybir
from concourse.tile import TileContext
from concourse.bass_utils import run_bass_kernel_spmd

B, C, CO, L, K = 16, 64, 64, 16384, 3
NCORES = 8
BLOC = B // NCORES          # 2 batches per core
HALO = 8
LP = L + 2 * HALO
MMN = 512                   # matmul free-dim chunk (one PSUM bank of fp32)
SEG = 128                   # smallside seg-layout: l = p*SEG + j
LB = L // 2                 # l_blk size (8192)
LBH = LB + 2 * HALO         # halo'd l_in extent per l_blk row
LC = 1024                   # product chunk along l_in
BF = mybir.dt.bfloat16
F32 = mybir.dt.float32
BF_NP = ml_dtypes.bfloat16

_BUILD_CACHE = {}


def _build():
    if "nc" in _BUILD_CACHE:
        return _BUILD_CACHE["nc"]
    nc = bacc.Bacc("TRN2")

    x2h = nc.dram_tensor("x2h", [2 * C, LP], BF, kind="ExternalInput")
    x2r = nc.dram_tensor("x2r", [128, 2 * LBH], BF, kind="ExternalInput")
    w_br = nc.dram_tensor("w_br", [2 * C, 7 * 6], BF, kind="ExternalInput")
    b_br = nc.dram_tensor("b_br", [128, 6], F32, kind="ExternalInput")
    w_vr = nc.dram_tensor("w_vr", [128, 2 * K * CO], BF, kind="ExternalInput")
    b_out = nc.dram_tensor("b_out", [CO, 1], F32, kind="ExternalInput")
    out = nc.dram_tensor("out", [BLOC, CO, L], F32, kind="ExternalOutput")

    br_dram = nc.dram_tensor("br_scratch", [BLOC, 6, L], F32)
    mw_dram = nc.dram_tensor("mw_scratch", [BLOC, 9, L], BF)

    with TileContext(nc) as tc:
        with (
            tc.tile_pool(name="outer", bufs=1) as bigp,
            tc.tile_pool(name="const", bufs=1) as constp,
            tc.tile_pool(name="osb", bufs=3) as outp,
        ):
            x2r_sb = bigp.tile([128, 2 * LBH], BF, tag="x2r")
            nc.sync.dma_start(out=x2r_sb[:], in_=x2r[:])
            wvr_sb = constp.tile([128, 2 * K * CO], BF, tag="wvr")
            nc.sync.dma_start(out=wvr_sb[:], in_=w_vr[:])
            bout_sb = constp.tile([CO, 1], F32, tag="bout")
            nc.sync.dma_start(out=bout_sb[:], in_=b_out[:])

            # ======== phase 1: branch convs + smallside (scoped pools) ========
            with (
                tc.tile_pool(name="ph1", bufs=1) as p1p,
                tc.tile_pool(name="seg", bufs=1) as segp,
                tc.tile_pool(name="ps_br", bufs=3, space="PSUM") as psbrp,
            ):
                xsb = p1p.tile([2 * C, LP], BF, tag="x")
                nc.sync.dma_start(out=xsb[:], in_=x2h[:])
                wbr_sb = constp.tile([2 * C, 7 * 6], BF, tag="wbr")
                nc.sync.dma_start(out=wbr_sb[:], in_=w_br[:])
                bbr_sb = constp.tile([128, 6], F32, tag="bbr")
                nc.sync.dma_start(out=bbr_sb[:], in_=b_br[:])

                brsb = p1p.tile([38, L], F32, tag="brsb")
                for i in range(L // MMN):
                    pss = [psbrp.tile([6, MMN], F32, tag=f"psbr{b}", name=f"psbr{b}")
                           for b in range(BLOC)]
                    for t in range(7):
                        for b in range(BLOC):
                            nc.tensor.matmul(
                                pss[b][:],
                                lhsT=wbr_sb[b * C:(b + 1) * C, t * 6:(t + 1) * 6],
                                rhs=xsb[b * C:(b + 1) * C,
                                        HALO + i * MMN + (t - 3): HALO + (i + 1) * MMN + (t - 3)],
                                start=(t == 0), stop=(t == 6),
                            )
                    for b in range(BLOC):
                        dst = brsb[b * 32:b * 32 + 6, i * MMN:(i + 1) * MMN]
                        if b == 0:
                            nc.vector.tensor_copy(out=dst, in_=pss[b][:])
                        else:
                            nc.scalar.copy(out=dst, in_=pss[b][:])
                for b in range(BLOC):
                    nc.sync.dma_start(out=br_dram[b, :, :],
                                      in_=brsb[b * 32:b * 32 + 6, :])

                # ---- smallside in seg layout [128, (b, f, j)] ----
                nseg = L // SEG  # 128
                brseg = segp.tile([nseg, BLOC * 6 * SEG], F32, tag="brseg")
                nc.sync.dma_start(
                    out=brseg[:].rearrange("p (b f j) -> p b f j", b=BLOC, f=6),
                    in_=br_dram[:].rearrange("b f (p j) -> p b f j", j=SEG),
                )
                brv = brseg[:].rearrange("p (b f j) -> p b f j", b=BLOC, f=6)
                nc.vector.tensor_tensor(
                    out=brv, in0=brv,
                    in1=bbr_sb[:].unsqueeze(1).unsqueeze(3).to_broadcast(
                        (nseg, BLOC, 6, SEG)),
                    op=mybir.AluOpType.add,
                )
                mskseg = segp.tile([nseg, BLOC * 3 * SEG], F32, tag="mskseg")
                am1 = segp.tile([nseg, BLOC * 3 * SEG], F32, tag="am1")
                ap1 = segp.tile([nseg, BLOC * 3 * SEG], F32, tag="ap1")
                a0 = segp.tile([nseg, BLOC * 3 * SEG], F32, tag="a0")
                for b in range(BLOC):
                    dst = slice(b * 3 * SEG, (b + 1) * 3 * SEG)
                    nc.scalar.activation(
                        out=mskseg[:, dst].rearrange("p (g j) -> p g j", j=SEG),
                        in_=brv[:, b, 3:6, :],
                        func=mybir.ActivationFunctionType.Sigmoid)
                    nc.scalar.activation(
                        out=am1[:, dst].rearrange("p (g j) -> p g j", j=SEG),
                        in_=brv[:, b, 0:3, :],
                        func=mybir.ActivationFunctionType.Relu, scale=-1.0)
                    nc.scalar.activation(
                        out=ap1[:, dst].rearrange("p (g j) -> p g j", j=SEG),
                        in_=brv[:, b, 0:3, :],
                        func=mybir.ActivationFunctionType.Relu)
                nc.vector.tensor_tensor(out=a0[:], in0=am1[:], in1=ap1[:],
                                        op=mybir.AluOpType.add)
                nc.vector.tensor_scalar(out=a0[:], in0=a0[:], scalar1=-1.0,
                                        scalar2=1.0, op0=mybir.AluOpType.mult,
                                        op1=mybir.AluOpType.add)
                mwseg = segp.tile([nseg, BLOC * 9 * SEG], BF, tag="mwseg")
                mwv = mwseg[:].rearrange("p (b d g) -> p b d g", b=BLOC, d=3)
                for b in range(BLOC):
                    sl = slice(b * 3 * SEG, (b + 1) * 3 * SEG)
                    for d_idx, t in enumerate((am1, a0, ap1)):
                        nc.vector.tensor_tensor(out=mwv[:, b, d_idx, :],
                                                in0=mskseg[:, sl], in1=t[:, sl],
                                                op=mybir.AluOpType.mult)
                nc.sync.dma_start(
                    out=mw_dram[:].rearrange("b f (p j) -> p b f j", j=SEG),
                    in_=mwseg[:].rearrange("p (b f j) -> p b f j", b=BLOC, f=9),
                )

            # ======== phase 2: products (R32 layout) + val matmuls ========
            ph2 = (
                tc.tile_pool(name="mw", bufs=2),
                tc.tile_pool(name="val", bufs=2),
                tc.tile_pool(name="tmp", bufs=2),
                tc.tile_pool(name="ps_o", bufs=2, space="PSUM"),
            )
            mwp, valp, tmpp, psop = [p.__enter__() for p in ph2]
            for ci in range(LB // LC):
                j0 = HALO + ci * LC
                mw2 = []
                for f in range(9):
                    t = mwp.tile([128, LC], BF, tag=f"mw2_{f}", name=f"mw2_{f}")
                    for b in range(BLOC):
                        for g in range(2):
                            eng = nc.sync if (f % 2 == 0) else nc.scalar
                            eng.dma_start(
                                out=t[b * 64 + g * 32: b * 64 + (g + 1) * 32, :],
                                in_=mw_dram[b, f:f + 1,
                                            g * LB + ci * LC: g * LB + (ci + 1) * LC
                                            ].partition_broadcast(32),
                            )
                    mw2.append(t)

                def xv(shift):
                    return x2r_sb[:].rearrange("r (c j) -> r c j", c=2)[
                        :, :, j0 + shift: j0 + shift + LC]

                vals = []
                for k in range(K):
                    vk = valp.tile([128, 2 * LC], BF, tag=f"val_{k}", name=f"val_{k}")
                    vkv = vk[:].rearrange("r (c j) -> r c j", c=2)
                    tmp = tmpp.tile([128, 2 * LC], BF, tag="ptmp", name="ptmp")
                    tmpv = tmp[:].rearrange("r (c j) -> r c j", c=2)
                    nc.vector.tensor_tensor(
                        out=vkv, in0=xv(k - 2),
                        in1=mw2[0 * 3 + k][:].unsqueeze(1).to_broadcast((128, 2, LC)),
                        op=mybir.AluOpType.mult)
                    nc.vector.tensor_tensor(
                        out=tmpv, in0=xv(k - 1),
                        in1=mw2[1 * 3 + k][:].unsqueeze(1).to_broadcast((128, 2, LC)),
                        op=mybir.AluOpType.mult)
                    nc.vector.tensor_tensor(out=vk[:], in0=vk[:], in1=tmp[:],
                                            op=mybir.AluOpType.add)
                    nc.vector.tensor_tensor(
                        out=tmpv, in0=xv(k),
                        in1=mw2[2 * 3 + k][:].unsqueeze(1).to_broadcast((128, 2, LC)),
                        op=mybir.AluOpType.mult)
                    nc.vector.tensor_tensor(out=vk[:], in0=vk[:], in1=tmp[:],
                                            op=mybir.AluOpType.add)
                    vals.append(vk)

                osbs = [outp.tile([CO, LC], F32, tag=f"osb{g}", name=f"osb{g}")
                        for g in range(4)]
                for nh in range(LC // MMN):
                    psos = [psop.tile([CO, MMN], F32, tag=f"pso{g}", name=f"pso{g}")
                            for g in range(4)]
                    for k in range(K):
                        for cl in range(2):
                            for g in range(4):
                                rb = g * 32
                                nc.tensor.matmul(
                                    psos[g][:],
                                    lhsT=wvr_sb[rb:rb + 32,
                                                (cl * K + k) * CO:(cl * K + k + 1) * CO],
                                    rhs=vals[k][rb:rb + 32,
                                                cl * LC + nh * MMN: cl * LC + (nh + 1) * MMN],
                                    start=(k == 0 and cl == 0),
                                    stop=(k == K - 1 and cl == 1),
                                    tile_position=(rb, 0),
                                )
                    for g in range(4):
                        nc.scalar.activation(
                            out=osbs[g][:, nh * MMN:(nh + 1) * MMN], in_=psos[g][:],
                            func=mybir.ActivationFunctionType.Identity,
                            bias=bout_sb[:], scale=1.0)
                for g in range(4):
                    b, lb = g // 2, g % 2
                    l0 = lb * LB + ci * LC
                    nc.sync.dma_start(out=out[b, :, l0:l0 + LC], in_=osbs[g][:])

            for p in reversed(ph2):
                p.__exit__(None, None, None)

    nc.compile()
    _BUILD_CACHE["nc"] = nc
    return nc


def _host_prep(inputs):
    x = np.asarray(inputs["x"], np.float32)
    w_off_dw = np.asarray(inputs["w_off_dw"], np.float32)
    b_off_dw = np.asarray(inputs["b_off_dw"], np.float32)
    w_off_pw = np.asarray(inputs["w_off_pw"], np.float32)
    b_off_pw = np.asarray(inputs["b_off_pw"], np.float32)
    w_msk_dw = np.asarray(inputs["w_msk_dw"], np.float32)
    b_msk_dw = np.asarray(inputs["b_msk_dw"], np.float32)
    w_msk_pw = np.asarray(inputs["w_msk_pw"], np.float32)
    b_msk_pw = np.asarray(inputs["b_msk_pw"], np.float32)
    weight = np.asarray(inputs["weight"], np.float32)
    bias = np.asarray(inputs["bias"], np.float32)

    wf = np.zeros((C, 7, 6), np.float32)
    wf[:, :, 0:3] = (w_off_pw[:, :, 0].T[:, None, :] * w_off_dw[:, 0, :][:, :, None])
    wf[:, :, 3:6] = (w_msk_pw[:, :, 0].T[:, None, :] * w_msk_dw[:, 0, :][:, :, None])
    beff = np.zeros(6, np.float32)
    beff[0:3] = b_off_pw + w_off_pw[:, :, 0] @ b_off_dw
    beff[3:6] = b_msk_pw + w_msk_pw[:, :, 0] @ b_msk_dw

    w_br = np.vstack([wf.reshape(C, 42)] * 2).astype(BF_NP)
    b_br = np.broadcast_to(beff[None, :], (128, 6)).copy()
    # w_vr[r, (c_lo, k, o)] = weight[o, c_hi*2+c_lo, k], r = b*64+l_blk*32+c_hi
    wv = weight.transpose(1, 2, 0).reshape(32, 2, K, CO)      # [c_hi, c_lo, k, o]
    wv = wv.reshape(32, 2 * K * CO)                           # [c_hi, (c_lo k o)]
    w_vr = np.vstack([wv] * 4).astype(BF_NP)                  # 4 row-groups
    b_out = bias.reshape(CO, 1).copy()

    in_maps = []
    for core in range(NCORES):
        xb = x[core * BLOC:(core + 1) * BLOC]                 # [2, C, L]
        xbf = xb.astype(BF_NP)
        x2h = np.zeros((2 * C, LP), BF_NP)
        x2h[:, HALO:HALO + L] = xbf.reshape(2 * C, L)
        # x2r[b, l_blk, c_hi, c_lo, j] = x[b, c_hi*2+c_lo, l_blk*LB + j - HALO]
        x2r = np.zeros((2, 2, 32, 2, LBH), BF_NP)
        xr = xbf.reshape(2, 32, 2, 2, LB)                     # [b, c_hi, c_lo, l_blk, l_in]
        xr = xr.transpose(0, 3, 1, 2, 4)                      # [b, l_blk, c_hi, c_lo, l_in]
        x2r[:, :, :, :, HALO:HALO + LB] = xr
        x2r[:, 1, :, :, :HALO] = xr[:, 0, :, :, LB - HALO:]
        x2r[:, 0, :, :, HALO + LB:] = xr[:, 1, :, :, :HALO]
        in_maps.append({
            "x2h": x2h, "x2r": x2r.reshape(128, 2 * LBH),
            "w_br": w_br, "b_br": b_br, "w_vr": w_vr, "b_out": b_out,
        })
    return in_maps


def kernel(**inputs):
    nc = _build()
    in_maps = _host_prep(inputs)
    res = run_bass_kernel_spmd(nc, in_maps, list(range(NCORES)))
    out = np.empty((B, CO, L), np.float32)
    for core in range(NCORES):
        out[core * BLOC:(core + 1) * BLOC] = res.results[core]["out"]
    return out
